# revision 96
# baseline (speedup 1.0000x reference)
"""Longformer block on 8 TRN2 NeuronCores (Bass/Tile, SPMD).

Sharding: data-parallel over (batch, sequence): core c -> batch c//4, token
chunk (c%4)*512..+512. Weights replicated. Everything on-chip stays in
transposed [D, token] layout so LN/residual/matmuls need no device transposes
(host pre-transposes x; LN stats via ones-vector matmuls on PE).

QKV, out-proj, and the FULL FFN run as fp8e4 DoubleRow matmuls. The FFN
weights are host-split hi/lo (lo = fp8 quantization residual at the SAME
scale, so hi+lo share one PSUM group and dequant): 2x fp8 throughput at
~bf16/2 error. FFN weights stream through double-buffered pools (each
column block is read exactly once). Both LayerNorms are folded: gains into
the downstream weight rows (host), biases into evacuation biases / gelu
bias / bo (V's ln1_b share is constant post-softmax, so it folds into bo);
on device LN is just ones-matmul stats + a broadcast (x-muB)*rstdB pair
writing fp8 directly. LN2 stats run inline under the Wo evacuations.

Attention is key-major: scores are computed transposed [keys, queries] so
the exp output feeds PV directly (no P transposes), normalization is
deferred (PV on raw exp, one reciprocal-broadcast multiply per pr block),
and iteration k+1's score matmuls are emitted before iteration k's
exp-dependent matmuls so the in-order PE never idles under the exp latency.
The token-0 global column is one batched query-major score pass for all 16
heads. The global *row* (token T-1) is combined across cores via a tiny
AllReduce and patched with copy_predicated (unchanged from baseline).

PSUM matmul regions each need their own start/stop bracket; matmul
stationary+dst base partitions must be 0/32/64 (DR dst: 0). Engine writes
must start at partition 0/32/64.
"""

import numpy as np
import ml_dtypes

import concourse.bass as bass
import concourse.mybir as mybir
import concourse.tile as tile
from concourse.masks import make_identity
from concourse.bass_utils import run_bass_kernel_spmd

F32 = mybir.dt.float32
BF16 = mybir.dt.bfloat16
FP8 = mybir.dt.float8e4
AF = mybir.ActivationFunctionType
ALU = mybir.AluOpType
AX = mybir.AxisListType
DR = mybir.MatmulPerfMode.DoubleRow

D = 1024
H = 16
HD = 64
T = 2048
B = 2
CHUNK = 512
HALO = 128
NSLOT = 768          # [halo 128 | own 512 | t0 | t2047 | pad]
NKV = 641            # slots 0..640 hold K/V (640 = token0); 641 = q2047 src
NQB = 4
WIN = 256
NEG = -1e30
EPS = 1e-5
N_CORES = 8
SW = 2048.0          # host weight scale for fp8 (Wq/Wk/Wv/Wo)
SH = 16.0            # activation scale for fp8 (hT, OT, V8, P)
SKIP_CC = [False]   # set kernel.SKIP_CC[0]=True to build without the
                    # collective (TimelineSim is single-core only)
PHASE_MARKS = []    # (phase_name, first_inst_id) filled during _emit


def _mark(nc, name):
    PHASE_MARKS.append((name, set(nc.inst_map.keys())))

# ---------------------------------------------------------------- bir fix ---

_waitfix_ctr = [0]


def _split_multiwaits(nc):
    """This container's walrus accepts ONE sync-wait per instruction; Tile
    attaches several. Hoist extras onto NoOps just before each instruction
    (Tile sems are monotonic within a context, so sequential waits are
    equivalent)."""
    n = 0
    for func in nc.m.functions:
        for bb in func.blocks:
            out = []
            changed = False
            for inst in bb.instructions:
                si = inst.sync_info
                if si is not None and len(si.on_wait) > 1:
                    waits = list(si.on_wait)
                    keep = [w for w in waits
                            if getattr(w, "wait_mode", "") not in
                            ("sem-ge-imm", "sem-ge-reg")]
                    if keep:
                        hoist = [w for w in waits if w not in keep]
                        last = keep
                    else:
                        hoist, last = waits[:-1], [waits[-1]]
                    for w in hoist:
                        _waitfix_ctr[0] += 1
                        nop = mybir.InstNoOp(name=f"I-waitfix-{_waitfix_ctr[0]}")
                        nop.engine = inst.engine
                        nop.sync_info = mybir.SyncInfo(on_wait=[w], on_update=[])
                        out.append(nop)
                        n += 1
                    si.on_wait = last
                    changed = True
                out.append(inst)
            if changed:
                bb.instructions[:] = out
    return n

# ------------------------------------------------------------ host helpers --


def _make_x_ext(x, c):
    b, j = divmod(c, 4)
    start = j * CHUNK
    ext = np.zeros((NSLOT, D), np.float32)
    ext[0:HALO] = x[b, start - HALO:start] if j > 0 else x[b, 0:HALO]
    ext[HALO:HALO + CHUNK] = x[b, start:start + CHUNK]
    ext[640] = x[b, 0]
    ext[641] = x[b, T - 1]
    return ext


def _make_mask(c):
    b, j = divmod(c, 4)
    start = j * CHUNK
    m = np.full((NQB, 128, WIN + 1), NEG, np.float32)
    il = np.arange(128)[:, None]
    jl = np.arange(WIN)[None, :]
    for qb in range(NQB):
        q_abs = start + qb * 128 + il
        slot = qb * 128 + jl
        band = (jl >= il) & (jl <= il + 128)
        valid = (j > 0) | (slot >= HALO)
        blk = m[qb, :, :WIN]
        blk[band & valid] = 0.0
        tok0_in_band = (q_abs[:, 0] <= HALO) & (j == 0)
        m[qb, :, WIN] = np.where(tok0_in_band, NEG, 0.0)
    return m


def _tileP(a, p=128):
    """[N*p, ...] -> [p, N, ...] partition-tiled layout."""
    n = a.shape[0] // p
    return np.ascontiguousarray(
        a.reshape(n, p, *a.shape[1:]).transpose(1, 0, *range(2, a.ndim + 1)))


def _vec_t(v):
    return np.ascontiguousarray(np.asarray(v, np.float32).reshape(-1, 128).T)

# ------------------------------------------------------------ bass program --


def _build_nc():
    nc = bass.Bass()

    inp = {}
    for name, shape, dt in [
        ("xT", [128, 8, NSLOT], BF16),
        ("wq", [128, 8, D], FP8), ("wk", [128, 8, D], FP8),
        ("wv", [128, 8, D], FP8), ("wo", [128, 8, D], FP8),
        ("w1hl", [128, 8, 16, 512], FP8), ("w2hl", [128, 8, 64, 128], FP8),
        ("msk", [128, NQB, 2, 128], BF16), ("mskg", [128, NQB], F32),
        ("pblob", [128, 84], F32),
        ("fixsel", [128, 1], mybir.dt.uint8),
    ]:
        inp[name] = nc.dram_tensor(name, shape, dt, kind="ExternalInput")
    out_d = nc.dram_tensor("outT", [128, 8, CHUNK], F32, kind="ExternalOutput")
    pin = nc.dram_tensor("pin", [H, 2, HD + 1], F32)
    pout = nc.dram_tensor("pout", [H, 2, HD + 1], F32, addr_space="Shared")

    with tile.TileContext(nc) as tc:
        _emit(nc, tc, inp, out_d, pin, pout)
    _split_multiwaits(nc)
    return nc


def _emit(nc, tc, inp, out_d, pin, pout):
    from contextlib import ExitStack
    ctx = ExitStack()
    with ctx:
        pers = ctx.enter_context(tc.tile_pool(name="pers", bufs=1))
        small = ctx.enter_context(tc.tile_pool(name="small", bufs=3))
        bigY = ctx.enter_context(tc.tile_pool(name="bigY", bufs=1))  # yT

        # ---- persistent constants / params
        idf = pers.tile([128, 128], F32, tag="idf")
        make_identity(nc, idf)
        idb = pers.tile([128, 128], BF16, tag="idb")
        make_identity(nc, idb)
        onesD = pers.tile([128, 1], BF16, tag="onesD")   # 1/D for means
        nc.vector.memset(onesD, 1.0 / D)
        onesDf = pers.tile([128, 1], F32, tag="onesDf")
        nc.vector.memset(onesDf, 1.0 / D)
        ones8 = pers.tile([128, 1], FP8, tag="ones8")
        nc.vector.memset(ones8, 1.0)
        ones16 = pers.tile([1, 128], F32, tag="ones16")  # LN1 bcast (x16 fp8)
        nc.vector.memset(ones16, SH)
        ones1f = pers.tile([1, 128], F32, tag="ones1f")  # LN2 bcast
        nc.vector.memset(ones1f, 1.0)
        epst = pers.tile([1, 1], F32, tag="epst")
        nc.vector.memset(epst, EPS)
        neg3 = pers.tile([128, 1], F32, tag="neg3")
        nc.vector.memset(neg3, -3.0)

        yT = bigY.tile([128, 8, CHUNK], F32, tag="yT")
        h2T8 = bigY.tile([128, 8, CHUNK], FP8, tag="h2T8")

        # ================= LN in transposed layout =========================
        def layernorm_T(src, xbt, width, nchunks, g, b, ones_bc, out, pools,
                        apply_width=None):
            apply_width = apply_width or width
            ps_row, ps_bc = pools
            cw = width // nchunks
            mus = []
            for cch in range(nchunks):
                mus.append((ps_row.tile([1, cw], F32, tag="row", name="mu"),
                            ps_row.tile([1, cw], F32, tag="row", name="msq")))
            for kt in range(8):
                xsq = small.tile([128, width], BF16, tag="ln_xsq")
                if xbt is None:
                    xb = src[:, kt, 0:width]
                    # bf16 source: square on DVE (2x mode), Act stays free
                    nc.vector.tensor_mul(out=xsq, in0=xb, in1=xb)
                else:
                    xb = xbt[:, kt, 0:width]
                    # cast copy split DVE/Pool; feeds only the apply stage
                    if kt % 2 == 0:
                        nc.vector.tensor_copy(out=xb, in_=src[:, kt, :])
                    else:
                        nc.gpsimd.tensor_copy(out=xb, in_=src[:, kt, :])
                    nc.scalar.square(out=xsq, in_=src[:, kt, :])
                for cch in range(nchunks):
                    sl = slice(cch * cw, (cch + 1) * cw)
                    nc.tensor.matmul(mus[cch][0], onesD, xb[:, sl],
                                     start=kt == 0, stop=kt == 7)
                    nc.tensor.matmul(mus[cch][1], onesD, xsq[:, sl],
                                     start=kt == 0, stop=kt == 7)
            bcs = []
            for cch in range(nchunks):
                mu_ps, msq_ps = mus[cch]
                musb = small.tile([1, cw], F32, tag="ln_mu")
                nc.scalar.copy(out=musb, in_=mu_ps)
                tmp = small.tile([1, cw], F32, tag="ln_tmp")
                nc.vector.tensor_mul(out=tmp, in0=musb, in1=musb)
                nc.vector.tensor_sub(out=tmp, in0=msq_ps, in1=tmp)
                nc.scalar.activation(out=tmp, in_=tmp, func=AF.Sqrt,
                                     bias=epst, scale=1.0)
                nc.vector.reciprocal(out=tmp, in_=tmp)       # rstd
                nc.vector.tensor_mul(out=musb, in0=musb, in1=tmp)
                nc.scalar.mul(out=musb, in_=musb, mul=-1.0)  # -mu*rstd
                rb_ps = ps_bc.tile([128, cw], F32, tag="bc", name="rb")
                nc.tensor.matmul(rb_ps, ones_bc, tmp, start=True, stop=True)
                nb_ps = ps_bc.tile([128, cw], F32, tag="bc", name="nb")
                nc.tensor.matmul(nb_ps, ones_bc, musb, start=True, stop=True)
                rb_sb = small.tile([128, cw], BF16, tag="ln_rb")
                nc.scalar.copy(out=rb_sb, in_=rb_ps)
                nb_sb = small.tile([128, cw], BF16, tag="ln_nb")
                nc.scalar.copy(out=nb_sb, in_=nb_ps)
                bcs.append((rb_sb, nb_sb))
            for kt in range(8):
                for cch in range(nchunks):
                    lo, hi = cch * cw, min((cch + 1) * cw, apply_width)
                    if hi <= lo:
                        continue
                    w = hi - lo
                    sl = slice(lo, hi)
                    rb_sb, nb_sb = bcs[cch]
                    src_kt = (src[:, kt, :] if xbt is None
                              else xbt[:, kt, 0:width])
                    t1 = small.tile([128, cw], BF16, tag="ln_t1")
                    t1 = t1[:, 0:w]
                    nc.vector.tensor_mul(out=t1, in0=src_kt[:, sl],
                                         in1=rb_sb[:, 0:w])
                    nc.vector.tensor_add(out=t1, in0=t1, in1=nb_sb[:, 0:w])
                    # g,b apply + dtype cast on Act
                    nc.scalar.activation(out=out[:, kt, sl], in_=t1,
                                         func=AF.Identity,
                                         bias=b[:, kt:kt + 1],
                                         scale=g[:, kt:kt + 1])

        with tc.tile_pool(name="bigG", bufs=1) as bigG:
            qctx = ExitStack()
            poolW = qctx.enter_context(tc.tile_pool(name="poolW", bufs=1))
            poolB = qctx.enter_context(tc.tile_pool(name="poolB", bufs=1))
            # xT first in the DMA queue (LN1 is the startup critical path)
            xT = bigG.tile([128, 8, NSLOT], BF16, tag="xT")
            for kt in range(8):
                nc.sync.dma_start(out=xT[:, kt, :], in_=inp["xT"][:, kt, :])

            pblob = pers.tile([128, 84], F32, tag="pblob")
            nc.sync.dma_start(out=pblob, in_=inp["pblob"][:])
            # packed params: [vbq vbk](8 each) free(16) bo bo2 b1h(32) fA fB
            vbqT = pblob[:, 0:8]      # Wq^T ln1_b / sqrt(hd)
            vbkT = pblob[:, 8:16]     # Wk^T ln1_b
            boT = pblob[:, 32:40]
            bo2T = pblob[:, 40:48]
            b1h = pblob[:, 48:80]
            fA = pblob[0:16, 80:81]
            fB = pblob[0:16, 81:82]
            fixsel = pers.tile([128, 1], mybir.dt.uint8, tag="fixsel")
            nc.sync.dma_start(out=fixsel, in_=inp["fixsel"][:])
            msk = pers.tile([128, NQB, 2, 128], BF16, tag="msk")
            nc.sync.dma_start(out=msk, in_=inp["msk"][:])

            # fp8 projection weights (whole tensors, one DMA each)
            wq8 = poolW.tile([128, 8, D], FP8, tag="wq8")
            nc.sync.dma_start(out=wq8, in_=inp["wq"][:])
            wk8 = poolW.tile([128, 8, D], FP8, tag="wk8")
            nc.sync.dma_start(out=wk8, in_=inp["wk"][:])
            wv8 = poolW.tile([128, 8, D], FP8, tag="wv8")
            nc.sync.dma_start(out=wv8, in_=inp["wv"][:])
            wo8 = bigG.tile([128, 8, D], FP8, tag="wo8")
            nc.sync.dma_start(out=wo8, in_=inp["wo"][:])

            hT8 = poolB.tile([128, 8, NSLOT], FP8, tag="hT8")
            QT = bigG.tile([128, 8, CHUNK], BF16, tag="QT")
            q47T = bigG.tile([128, 8], BF16, tag="q47T")
            KT = bigG.tile([128, 8, NKV], BF16, tag="KT")
            V8 = bigG.tile([128, 5, D], FP8, tag="V8")    # x16 scale
            v0r = bigG.tile([1, D], BF16, tag="v0r")      # x16 scale
            # zero-padded per-(sub,pr) copies of v0 so the rank-1 token-0
            # update can always target PSUM partitions 0..127 (walrus
            # rejects K=1 matmuls with dst partition base 64)
            v0z = bigG.tile([1, 2, 8, 128], BF16, tag="v0z")
            OT8 = bigG.tile([128, 8, CHUNK], FP8, tag="OT8")  # x16 scale

            _mark(nc, "B:ln1")
            # ===== Phase B: LN1 folded -> hT8 = (x-mu)*rstd*16 (fp8) =======
            # g1 folded into Wq/Wk/Wv rows (host); ln1_b enters via the QKV
            # evacuation biases, so no gain/bias apply pass is needed.
            with tc.tile_pool(name="ps_row1", bufs=4, space="PSUM") as psr, \
                 tc.tile_pool(name="ps_bc1", bufs=4, space="PSUM") as psb:
                ones16b1 = pers.tile([1, 128], BF16, tag="ones16b1")
                nc.vector.memset(ones16b1, SH)
                ones1rb1 = pers.tile([1, 128], BF16, tag="ones1rb1")
                nc.vector.memset(ones1rb1, 1.0)
                mus = [(psr.tile([1, 384], F32, tag="row", name=f"mu{c}"),
                        psr.tile([1, 384], F32, tag="row", name=f"ms{c}"))
                       for c in range(2)]
                for kt in range(8):
                    xb = xT[:, kt, :]
                    xsq = small.tile([128, NSLOT], BF16, tag="ln_xsq")
                    nc.vector.tensor_mul(out=xsq, in0=xb, in1=xb)
                    for cch in range(2):
                        sl = slice(cch * 384, (cch + 1) * 384)
                        nc.tensor.matmul(mus[cch][0], onesD, xb[:, sl],
                                         start=kt == 0, stop=kt == 7)
                        nc.tensor.matmul(mus[cch][1], onesD, xsq[:, sl],
                                         start=kt == 0, stop=kt == 7)
                rB = pers.tile([128, 2, 384], BF16, tag="ln_rb")
                mB = pers.tile([128, 2, 384], BF16, tag="ln_nb")
                for cch in range(2):
                    mu_ps, msq_ps = mus[cch]
                    mu_b = small.tile([1, 384], BF16, tag="ln_mu")
                    nc.scalar.copy(out=mu_b, in_=mu_ps)
                    var = small.tile([1, 384], F32, tag="ln_tmp")
                    nc.vector.tensor_mul(out=var, in0=mu_b, in1=mu_b)
                    nc.vector.tensor_sub(out=var, in0=msq_ps, in1=var)
                    nc.scalar.activation(out=var, in_=var, func=AF.Sqrt,
                                         bias=epst, scale=1.0)
                    rstd_b = small.tile([1, 384], BF16, tag="ln_rs")
                    with nc.allow_low_precision(reason="LN bcast bf16"):
                        nc.vector.reciprocal(out=rstd_b, in_=var)
                    rB_ps = psb.tile([128, 384], F32, tag="bc", name="rB_ps")
                    nc.tensor.matmul(rB_ps, ones16b1, rstd_b,
                                     start=True, stop=True)
                    nc.scalar.copy(out=rB[:, cch, :], in_=rB_ps)
                    mB_ps = psb.tile([128, 384], F32, tag="bc", name="mB_ps")
                    nc.tensor.matmul(mB_ps, ones1rb1, mu_b,
                                     start=True, stop=True)
                    nc.scalar.copy(out=mB[:, cch, :], in_=mB_ps)
                mBv = mB.rearrange("p c w -> p (c w)")
                rBv = rB.rearrange("p c w -> p (c w)")
                for kt in range(8):
                    t1 = small.tile([128, 642], BF16, tag="ln_t1")
                    nc.vector.tensor_sub(out=t1, in0=xT[:, kt, 0:642],
                                         in1=mBv[:, 0:642])
                    if kt % 2 == 0:
                        nc.vector.tensor_mul(out=hT8[:, kt, 0:642],
                                             in0=t1, in1=rBv[:, 0:642])
                    else:
                        nc.gpsimd.tensor_mul(out=hT8[:, kt, 0:642],
                                             in0=t1, in1=rBv[:, 0:642])

            _mark(nc, "C:qkv")
            # ========= Phase C: QKV fp8 DoubleRow (+ q2047, v0 row) ========
            DQ = 1.0 / (SW * SH)
            with tc.tile_pool(name="ps_big", bufs=6, space="PSUM") as ps_big, \
                 tc.tile_pool(name="ps_tiny", bufs=2, space="PSUM") as ps_tiny:
                for tt in range(5):
                    for cch in range(2):
                        csl = slice(cch * 512, (cch + 1) * 512)
                        v_ps = ps_big.tile([128, 512], F32, tag="big")
                        for j in range(4):
                            pr = slice(2 * j, 2 * j + 2)
                            nc.tensor.matmul(
                                v_ps, hT8[:, pr, tt * 128:(tt + 1) * 128],
                                wv8[:, pr, csl],
                                start=j == 0, stop=j == 3, perf_mode=DR)
                        # V8 holds 16*v (fp8)
                        if (tt + cch) % 2 == 0:
                            nc.scalar.mul(out=V8[:, tt, csl], in_=v_ps,
                                          mul=DQ * SH)
                        else:
                            nc.vector.tensor_scalar_mul(
                                out=V8[:, tt, csl], in0=v_ps,
                                scalar1=DQ * SH)
                for m in range(8):
                    msl = slice(m * 128, (m + 1) * 128)
                    q_ps = ps_big.tile([128, CHUNK], F32, tag="big")
                    q47_ps = ps_tiny.tile([128, 1], F32, tag="tiny")
                    for j in range(4):
                        pr = slice(2 * j, 2 * j + 2)
                        nc.tensor.matmul(q_ps, wq8[:, pr, msl],
                                         hT8[:, pr, HALO:HALO + CHUNK],
                                         start=j == 0, stop=j == 3,
                                         perf_mode=DR)
                        nc.tensor.matmul(q47_ps, wq8[:, pr, msl],
                                         hT8[:, pr, 641:642],
                                         start=j == 0, stop=j == 3,
                                         perf_mode=DR)
                    nc.scalar.activation(out=QT[:, m, :], in_=q_ps,
                                          func=AF.Identity,
                                          bias=vbqT[:, m:m + 1],
                                          scale=DQ / np.sqrt(HD))
                    nc.scalar.activation(out=q47T[:, m:m + 1], in_=q47_ps,
                                         func=AF.Identity,
                                         bias=vbqT[:, m:m + 1],
                                         scale=DQ / np.sqrt(HD))
                    k_ps = ps_big.tile([128, 512], F32, tag="big")
                    k_ps2 = ps_big.tile([128, NKV - 512], F32, tag="big")
                    for j in range(4):
                        pr = slice(2 * j, 2 * j + 2)
                        nc.tensor.matmul(k_ps, wk8[:, pr, msl],
                                         hT8[:, pr, 0:512],
                                         start=j == 0, stop=j == 3,
                                         perf_mode=DR)
                        nc.tensor.matmul(k_ps2, wk8[:, pr, msl],
                                         hT8[:, pr, 512:NKV],
                                         start=j == 0, stop=j == 3,
                                         perf_mode=DR)
                    nc.vector.tensor_scalar(out=KT[:, m, 0:512],
                                            in0=k_ps, scalar1=DQ,
                                            scalar2=vbkT[:, m:m + 1],
                                            op0=ALU.mult, op1=ALU.add)
                    nc.vector.tensor_scalar(out=KT[:, m, 512:NKV],
                                            in0=k_ps2, scalar1=DQ,
                                            scalar2=vbkT[:, m:m + 1],
                                            op0=ALU.mult, op1=ALU.add)
                for cch in range(2):
                    csl = slice(cch * 512, (cch + 1) * 512)
                    v0_ps = ps_tiny.tile([1, 512], F32, tag="tiny")
                    for j in range(4):
                        pr = slice(2 * j, 2 * j + 2)
                        nc.tensor.matmul(v0_ps, hT8[:, pr, 640:641],
                                         wv8[:, pr, csl],
                                         start=j == 0, stop=j == 3,
                                         perf_mode=DR)
                    nc.scalar.mul(out=v0r[:, csl], in_=v0_ps, mul=DQ * SH)
                nc.gpsimd.memset(v0z, 0.0)
                v0v = v0r.rearrange("p (h c) -> p h c", c=128)
                nc.vector.tensor_copy(out=v0z[:, 0, :, 0:64],
                                      in_=v0v[:, :, 0:64])
                nc.vector.tensor_copy(out=v0z[:, 1, :, 64:128],
                                      in_=v0v[:, :, 64:128])

            # QKV weights + hT8 dead past this point
            qctx.close()

            def emit_D(ps_tiny):
                _mark(nc, "D:partials")
                # ========= Phase D: global-row partials + AllReduce ========
                s47_ps = ps_tiny.tile([128, H * 4], F32, tag="tiny")
                for h in range(H):
                    p0 = 64 * (h % 2)
                    for i in range(4):
                        nc.tensor.matmul(
                            s47_ps[:, 4 * h + i:4 * h + i + 1],
                            KT[p0:p0 + 64, h // 2,
                               HALO + 128 * i:HALO + 128 * (i + 1)],
                            q47T[p0:p0 + 64, h // 2:h // 2 + 1],
                            start=True, stop=True)
                p47 = small.tile([128, H * 4], FP8, tag="p_p47")
                nc.scalar.activation(out=p47, in_=s47_ps, func=AF.Exp)
                ssum_ps = ps_tiny.tile([1, H * 4], F32, tag="tiny")
                nc.tensor.matmul(ssum_ps, ones8, p47, start=True, stop=True)
                s_c = small.tile([1, H], F32, tag="p_sc")
                nc.vector.reduce_sum(
                    out=s_c, in_=ssum_ps.rearrange("p (h i) -> p h i", i=4),
                    axis=AX.X)
                oall = small.tile([65, H], F32, tag="p_oall")
                o47_ps = ps_tiny.tile([64, H], F32, tag="tiny")
                for h in range(H):
                    for i in range(4):
                        # V8 is 16*v: o partial comes out 16x, matching the
                        # x16 fp8 output scale of the patch column.
                        nc.tensor.matmul(o47_ps[:, h:h + 1],
                                         V8[:, 1 + i, 64 * h:64 * h + 64],
                                         p47[:, 4 * h + i:4 * h + i + 1],
                                         start=i == 0, stop=i == 3)
                nc.scalar.copy(out=oall[0:64, :], in_=o47_ps)
                nc.sync.dma_start(out=oall[64:65, :], in_=s_c)
                part_ps = ps_tiny.tile([H, 65], F32, tag="tiny")
                nc.tensor.transpose(part_ps, oall, idf[0:65, 0:65])
                part_sb = small.tile([H, 65], F32, tag="p_part")
                nc.scalar.copy(out=part_sb, in_=part_ps)
                pa = small.tile([H, 2, 65], F32, tag="p_pa")
                nc.vector.tensor_scalar_mul(out=pa[:, 0, :], in0=part_sb,
                                            scalar1=fA)
                nc.vector.tensor_scalar_mul(out=pa[:, 1, :], in0=part_sb,
                                            scalar1=fB)
                nc.sync.dma_start(out=pin[:], in_=pa)
                if not SKIP_CC[0]:
                    nc.gpsimd.collective_compute(
                        "AllReduce", ALU.add,
                        replica_groups=[[0, 1, 2, 3, 4, 5, 6, 7]],
                        ins=[pin[:]], outs=[pout[:]])
                gath = small.tile([H, 2, 65], F32, tag="p_gath")
                nc.sync.dma_start(out=gath,
                                  in_=(pin if SKIP_CC[0] else pout)[:])
                vA = small.tile([H, 65], F32, tag="p_vA")
                nc.vector.tensor_scalar_mul(out=vA, in0=gath[:, 0, :],
                                            scalar1=fA)
                vB = small.tile([H, 65], F32, tag="p_vB")
                nc.vector.tensor_scalar_mul(out=vB, in0=gath[:, 1, :],
                                            scalar1=fB)
                val = small.tile([H, 65], F32, tag="p_val")
                nc.vector.tensor_add(out=val, in0=vA, in1=vB)
                recS = small.tile([H, 1], F32, tag="p_recS")
                nc.vector.reciprocal(out=recS, in_=val[:, 64:65])
                a47 = small.tile([H, HD], F32, tag="p_a47")
                # o partial is 16x -> a47 lands at the x16 fp8 scale directly
                nc.vector.tensor_scalar_mul(out=a47, in0=val[:, 0:64],
                                            scalar1=recS)
                a47t_ps = ps_tiny.tile([HD, H], F32, tag="tiny")
                nc.tensor.transpose(a47t_ps, a47, idf[0:H, 0:H])
                a47T = small.tile([HD, H], FP8, tag="p_a47T")
                nc.scalar.copy(out=a47T, in_=a47t_ps)
                fix_sb = small.tile([128, 8], FP8, tag="p_fix")
                a47v = a47T.rearrange("p (t two) -> p t two", two=2)
                nc.sync.dma_start(out=fix_sb[0:64, :], in_=a47v[:, :, 0])
                nc.sync.dma_start(out=fix_sb[64:128, :], in_=a47v[:, :, 1])
                return fix_sb

            _mark(nc, "E:attn")
            # ===== Phase E: windowed attention (key-major scores) ==========
            # Scores are computed transposed [keys, queries] so the exp
            # output feeds PV directly (no P transposes / PSUM copies), and
            # softmax normalization is deferred to the evacuation:
            # OT8 = o_ps * recipB where recipB broadcasts 1/rowsum.
            # Token-0 global column: one batched [16,512] score matmul + exp
            # for all heads; rank-1 PV updates join the o_ps accumulation.
            with tc.tile_pool(name="ps_s", bufs=2, space="PSUM") as ps_s, \
                 tc.tile_pool(name="ps_rs", bufs=2, space="PSUM") as ps_rs, \
                 tc.tile_pool(name="ps_rb", bufs=1, space="PSUM") as ps_rb, \
                 tc.tile_pool(name="ps_tiny2", bufs=1, space="PSUM") as pst2, \
                 tc.tile_pool(name="ps_o", bufs=2, space="PSUM") as ps_o:
                # consts for rsum / broadcast. Engine writes must start at
                # partition 0/32/64, so the two per-sub rowsums live at
                # partitions 0 and 32 of a [33,...] tile whose middle rows
                # are memset once (sel33 zeros null them in the matmul).
                ones1b = pers.tile([128, 1], BF16, tag="ones1b")
                nc.vector.memset(ones1b, 1.0)
                one1x1b = pers.tile([1, 1], BF16, tag="one1x1b")
                nc.vector.memset(one1x1b, 1.0)
                sel33 = pers.tile([33, 128], BF16, tag="sel33")
                nc.vector.memset(sel33, 0.0)
                nc.vector.memset(sel33[0:1, 0:64], 1.0)
                nc.vector.memset(sel33[32:33, 64:128], 1.0)
                r33 = bigG.tile([33, 8, CHUNK], BF16, tag="r33")
                nc.gpsimd.memset(r33, 1.0)
                OTb = bigG.tile([128, 8, CHUNK], BF16, tag="OTb")
                # batched global-column scores, computed query-major (out
                # partitions = queries; matmul dst base must be 0/32/64) with
                # the token-0 mask + the -3 exp shift folded into the Act
                # bias, then PE-transposed to key-major pg [16, 512]
                KTg = bigG.tile([128, 8, 2], BF16, tag="KTg")
                nc.vector.memset(KTg, 0.0)
                nc.vector.tensor_copy(out=KTg[0:64, :, 0], in_=KT[0:64, :, 640])
                nc.vector.tensor_copy(out=KTg[64:128, :, 1],
                                      in_=KT[64:128, :, 640])
                mskg = pers.tile([128, NQB], F32, tag="mskg")
                nc.sync.dma_start(out=mskg, in_=inp["mskg"][:])
                pgT = bigG.tile([128, NQB, 16], BF16, tag="pgT")
                pg_ps = ps_rb.tile([16, 512], BF16, tag="rb", name="pg_ps")
                for qb in range(NQB):
                    qsl = slice(qb * 128, (qb + 1) * 128)
                    gs_ps = ps_s.tile([128, 16], F32, tag="s", name="gs_ps")
                    for pr in range(8):
                        nc.tensor.matmul(gs_ps[:, 2 * pr:2 * pr + 2],
                                         QT[:, pr, qsl], KTg[:, pr, :],
                                         start=True, stop=pr == 7,
                                         skip_group_check=True)
                    nc.scalar.activation(out=pgT[:, qb, :], in_=gs_ps,
                                         func=AF.Exp,
                                         bias=mskg[:, qb:qb + 1], scale=1.0)
                    nc.tensor.transpose(pg_ps[:, qsl], pgT[:, qb, :], idb)
                pg = bigG.tile([16, 512], BF16, tag="pg")
                nc.scalar.copy(out=pg, in_=pg_ps)
                # partition-0 copy so rank-1 PV matmuls get matching bases
                # (DRAM roundtrip: row-major order is well-defined there)
                pg_d = nc.dram_tensor("pg_scratch", [16, 512], BF16)
                pgf = bigG.tile([1, 16, 512], BF16, tag="pgf")
                nc.sync.dma_start(out=pg_d[:], in_=pg)
                nc.sync.dma_start(out=pgf,
                                  in_=pg_d[:].rearrange("h q -> () h q"))

                # main loop, software-pipelined: iteration k+1's score
                # matmuls are emitted before iteration k's exp-dependent
                # matmuls so the in-order PE never idles under the exp
                # latency. Rowsums accumulate into per-(pr,sub) [1,512] PSUM
                # rows; normalization is a short bulk pass per pr.
                def emit_scores(pr, qb):
                    qsl = slice(qb * 128, (qb + 1) * 128)
                    sT = ps_s.tile([128, 2, 2, 128], F32, tag="s")
                    for sub in range(2):
                        dsl = slice(64 * sub, 64 * sub + 64)
                        qs = QT[dsl, pr, qsl]
                        for blk in range(2):
                            ks = qb * 128 + blk * 128
                            nc.tensor.matmul(
                                sT[:, sub, blk, :],
                                KT[dsl, pr, ks:ks + 128], qs,
                                start=True, stop=False, skip_group_check=True)
                            nc.tensor.matmul(
                                sT[:, sub, blk, :], idb, msk[:, qb, blk, :],
                                start=False, stop=True, skip_group_check=True)
                    return sT

                sT_next = emit_scores(0, 0)
                for pr in range(8):
                    rs_prs = [ps_rs.tile([1, CHUNK], F32, tag="rs",
                                         name=f"rs{s}") for s in range(2)]
                    for qb in range(NQB):
                        qsl = slice(qb * 128, (qb + 1) * 128)
                        sT_cur = sT_next
                        if (pr, qb) != (7, NQB - 1):
                            sT_next = emit_scores(pr + (qb + 1) // NQB,
                                                  (qb + 1) % NQB)
                        o_ps = ps_o.tile([128, 128], F32, tag="o")
                        p8s = []
                        for sub in range(2):
                            p8 = small.tile([128, 2, 128], BF16, tag="a_p8")
                            nc.scalar.activation(out=p8, in_=sT_cur[:, sub],
                                                 func=AF.Exp,
                                                 bias=neg3, scale=1.0)
                            p8s.append(p8)
                            nc.tensor.matmul(rs_prs[sub][:, qsl], ones1b,
                                             p8[:, 0, :],
                                             start=True, stop=False,
                                             skip_group_check=True)
                            nc.tensor.matmul(rs_prs[sub][:, qsl], ones1b,
                                             p8[:, 1, :],
                                             start=False, stop=False,
                                             skip_group_check=True)
                            nc.tensor.matmul(rs_prs[sub][:, qsl],
                                             one1x1b,
                                             pgf[0:1, 2 * pr + sub, qsl],
                                             start=False, stop=True,
                                             skip_group_check=True)
                        # PV (plain; fp8 stationary x bf16 moving) + token-0
                        # rank-1 updates in the same accumulation
                        for sub in range(2):
                            p0 = 64 * sub
                            h2s = 2 * pr + sub
                            dv = slice(64 * h2s, 64 * h2s + 64)
                            nc.tensor.matmul(o_ps[p0:p0 + 64, :],
                                             V8[:, qb, dv], p8s[sub][:, 0, :],
                                             start=True, stop=False,
                                             skip_group_check=True)
                            nc.tensor.matmul(o_ps[p0:p0 + 64, :],
                                             V8[:, qb + 1, dv],
                                             p8s[sub][:, 1, :],
                                             start=False, stop=False,
                                             skip_group_check=True)
                        for sub in range(2):
                            h2s = 2 * pr + sub
                            nc.tensor.matmul(o_ps, v0z[:, sub, pr, :],
                                             pgf[0:1, h2s, qsl],
                                             start=False, stop=sub == 1,
                                             skip_group_check=True)
                        if qb % 2 == 0:
                            nc.vector.tensor_copy(out=OTb[:, pr, qsl],
                                                  in_=o_ps)
                        else:
                            nc.scalar.copy(out=OTb[:, pr, qsl], in_=o_ps)
                    # bulk normalize this pr block
                    with nc.allow_low_precision(
                            reason="softmax renorm tolerates bf16"):
                        for sub in range(2):
                            rr = r33[32 * sub:32 * sub + 1, pr, :]
                            nc.vector.reciprocal(out=rr, in_=rs_prs[sub])
                    R_ps = ps_rb.tile([128, CHUNK], F32, tag="rb", name="R_ps")
                    nc.tensor.matmul(R_ps, sel33, r33[:, pr, :],
                                     start=True, stop=True)
                    nc.vector.tensor_mul(out=OT8[:, pr, :],
                                         in0=OTb[:, pr, :], in1=R_ps)
                    if pr == 0:
                        fix_sb = emit_D(pst2)

            _mark(nc, "F:patch")
            # ================ Phase F: patch global row ====================
            for t in range(8):
                nc.vector.copy_predicated(out=OT8[:, t, CHUNK - 1:CHUNK],
                                          mask=fixsel,
                                          data=fix_sb[:, t:t + 1])

            # residual + bo precomputed off the critical path (runs under
            # the attention phase wall)
            xTb = bigG.tile([128, 8, CHUNK], BF16, tag="xTb")
            for m in range(8):
                nc.gpsimd.tensor_scalar_add(out=xTb[:, m, :],
                                            in0=xT[:, m, HALO:HALO + CHUNK],
                                            scalar1=boT[:, m:m + 1])

            import os as _os
            if _os.environ.get("DUMP_OT8"):
                for m in range(8):
                    omd = small.tile([128, CHUNK], F32, tag="ot8d")
                    nc.vector.tensor_copy(out=omd, in_=OT8[:, m, :])
                    nc.sync.dma_start(out=out_d[:, m, :], in_=omd)

            _mark(nc, "G:wo")
            # === Phase G: out-proj fp8 DR + residual + inline LN2 ==========
            # LN2 is folded here: per-block squares/casts run on Pool/Act
            # under the Wo matmuls, the stats matmuls are a short PE tail,
            # and the apply writes h2T8 (x16 fp8) directly. g2 is folded
            # into W1 and b2's contribution into the gelu bias (host).
            with tc.tile_pool(name="ps_g", bufs=3, space="PSUM") as ps_g, \
                 tc.tile_pool(name="ps_st", bufs=1, space="PSUM") as ps_st, \
                 tc.tile_pool(name="ps_bc2", bufs=2, space="PSUM") as ps_bc2:
                ones16b = pers.tile([1, 128], BF16, tag="ones16b")
                nc.vector.memset(ones16b, SH)
                ones1rb = pers.tile([1, 128], BF16, tag="ones1rb")
                nc.vector.memset(ones1rb, 1.0)
                sqs, xbs = [], []
                for m in range(8):
                    msl = slice(m * 128, (m + 1) * 128)
                    pr_ps = ps_g.tile([128, CHUNK], F32, tag="g")
                    for j in range(4):
                        prj = slice(2 * j, 2 * j + 2)
                        nc.tensor.matmul(pr_ps, wo8[:, prj, msl],
                                         OT8[:, prj, :],
                                         start=j == 0, stop=j == 3,
                                         perf_mode=DR)
                    nc.vector.scalar_tensor_tensor(
                        out=yT[:, m, :], in0=pr_ps, scalar=DQ,
                        in1=xTb[:, m, :], op0=ALU.mult, op1=ALU.add)
                    # QT / OTb are dead after attention: reuse as LN2
                    # stats scratch (bf16 cast + squares)
                    xb = QT[:, m, :]
                    nc.gpsimd.tensor_copy(out=xb, in_=yT[:, m, :])
                    sq = OTb[:, m, :]
                    nc.scalar.square(out=sq, in_=xb)
                    xbs.append(xb)
                    sqs.append(sq)
                mu_ps = ps_st.tile([1, CHUNK], F32, tag="st", name="mu_ps")
                msq_ps = ps_st.tile([1, CHUNK], F32, tag="st2", name="msq_ps")
                for m in range(8):
                    nc.tensor.matmul(mu_ps, onesD, xbs[m],
                                     start=m == 0, stop=m == 7)
                    nc.tensor.matmul(msq_ps, onesD, sqs[m],
                                     start=m == 0, stop=m == 7)
                mu_b = small.tile([1, CHUNK], BF16, tag="ln2mu")
                nc.scalar.copy(out=mu_b, in_=mu_ps)
                var = small.tile([1, CHUNK], F32, tag="ln2var")
                nc.vector.tensor_mul(out=var, in0=mu_b, in1=mu_b)
                nc.vector.tensor_sub(out=var, in0=msq_ps, in1=var)
                nc.scalar.activation(out=var, in_=var, func=AF.Sqrt,
                                     bias=epst, scale=1.0)
                rstd_b = small.tile([1, CHUNK], BF16, tag="ln2rs")
                with nc.allow_low_precision(reason="LN rstd bcast bf16"):
                    nc.vector.reciprocal(out=rstd_b, in_=var)
                rstdB_ps = ps_bc2.tile([128, CHUNK], F32, tag="bc2",
                                       name="rstdB_ps")
                nc.tensor.matmul(rstdB_ps, ones16b, rstd_b,
                                 start=True, stop=True)
                rstdB = pers.tile([128, CHUNK], BF16, tag="ln2rB")
                nc.scalar.copy(out=rstdB, in_=rstdB_ps)
                muB_ps = ps_bc2.tile([128, CHUNK], F32, tag="bc2",
                                     name="muB_ps")
                nc.tensor.matmul(muB_ps, ones1rb, mu_b, start=True, stop=True)
                _mark(nc, "H:ln2")
                for m in range(8):
                    t1 = small.tile([128, CHUNK], BF16, tag="ln2t1")
                    nc.vector.tensor_sub(out=t1, in0=yT[:, m, :], in1=muB_ps)
                    if m % 2 == 0:
                        nc.vector.tensor_mul(out=h2T8[:, m, :], in0=t1,
                                             in1=rstdB)
                    else:
                        nc.gpsimd.tensor_mul(out=h2T8[:, m, :], in0=t1,
                                             in1=rstdB)

        # bigG/poolW/poolB closed: attention-side SBUF freed for the FFN
        # hi/lo fp8 FFN weights (host-split; lo holds the quantization
        # residual at the SAME scale SW, so hi+lo passes share one PSUM
        # accumulation group and a single dequant). Each weight column block
        # is read exactly once, so stream through double-buffered pools.
        with tc.tile_pool(name="poolF", bufs=1) as poolF, \
             tc.tile_pool(name="w1p", bufs=3) as w1p, \
             tc.tile_pool(name="w2p", bufs=3) as w2p:
            w1ts = []

            def w1_fetch():
                w1t = w1p.tile([128, 16, 512], FP8, tag="w1t")
                nc.sync.dma_start(out=w1t, in_=inp["w1hl"][:, len(w1ts)])
                w1ts.append(w1t)

            w1_fetch()   # 2 groups prefetched ahead of FFN1
            w1_fetch()
            htsb8 = poolF.tile([128, 32, CHUNK], FP8, tag="htsb8")

            _mark(nc, "I:ffn1")
            # ========= Phase I: FFN1 + gelu (hi/lo fp8 DoubleRow) ==========
            w2ts = []

            def w2_fetch():
                w2t = w2p.tile([128, 64, 128], FP8, tag="w2t")
                nc.sync.dma_start(out=w2t, in_=inp["w2hl"][:, len(w2ts)])
                w2ts.append(w2t)

            with tc.tile_pool(name="ps_f1", bufs=4, space="PSUM") as ps_f1:
                for mi in range(32):
                    g, gi = divmod(mi, 4)
                    if gi == 0 and len(w1ts) < 8:    # stream next W1 group
                        w1_fetch()
                    if mi in (25, 29):   # prefetch first 2 W2 blocks
                        w2_fetch()
                    msl = slice(gi * 128, (gi + 1) * 128)
                    h_ps = ps_f1.tile([128, CHUNK], F32, tag="f1")
                    for j in range(4):
                        prj = slice(2 * j, 2 * j + 2)
                        nc.tensor.matmul(h_ps, w1ts[g][:, prj, msl],
                                         h2T8[:, prj, :],
                                         start=j == 0, stop=False,
                                         perf_mode=DR)
                    for j in range(4):
                        plo = slice(8 + 2 * j, 8 + 2 * j + 2)
                        nc.tensor.matmul(h_ps, w1ts[g][:, plo, msl],
                                         h2T8[:, 2 * j:2 * j + 2, :],
                                         start=False, stop=j == 3,
                                         perf_mode=DR)
                    nc.scalar.activation(out=htsb8[:, mi, :], in_=h_ps,
                                         func=AF.Gelu,
                                         bias=b1h[:, mi:mi + 1], scale=DQ)

            _mark(nc, "J:ffn2")
            # ====== Phase J: FFN2 (hi/lo fp8 DR, streamed W2) + residual ===
            with tc.tile_pool(name="ps_f2", bufs=2, space="PSUM") as ps_f2:
                for m in range(8):
                    if m + 2 < 8:
                        w2_fetch()
                    w2t = w2ts[m]
                    f2_ps = ps_f2.tile([128, CHUNK], F32, tag="f2")
                    for j in range(16):
                        prj = slice(2 * j, 2 * j + 2)
                        nc.tensor.matmul(f2_ps, w2t[:, prj, :],
                                         htsb8[:, prj, :],
                                         start=j == 0, stop=False,
                                         perf_mode=DR)
                    for j in range(16):
                        plo = slice(32 + 2 * j, 32 + 2 * j + 2)
                        nc.tensor.matmul(f2_ps, w2t[:, plo, :],
                                         htsb8[:, 2 * j:2 * j + 2, :],
                                         start=False, stop=j == 15,
                                         perf_mode=DR)
                    om1 = small.tile([128, CHUNK], BF16, tag="out_m1")
                    nc.scalar.activation(out=om1, in_=f2_ps, func=AF.Identity,
                                         bias=bo2T[:, m:m + 1], scale=1.0 / SW)
                    om = small.tile([128, CHUNK], F32, tag="out_m")
                    import os as _os
                    if _os.environ.get("DUMP_Y"):
                        nc.vector.tensor_copy(out=om, in_=yT[:, m, :])
                    else:
                        nc.vector.tensor_add(out=om, in0=om1, in1=yT[:, m, :])
                    if not _os.environ.get("DUMP_OT8"):
                        nc.sync.dma_start(out=out_d[:, m, :], in_=om)

# ------------------------------------------------------------------ driver --

_CACHE = {}


def _prep_core_inputs(inputs, c, shared_cache={}):
    bf = ml_dtypes.bfloat16
    f8 = ml_dtypes.float8_e4m3
    key = id(inputs.get("Wq"))
    shared = shared_cache.get(key)
    if shared is None:
        shared_cache.clear()

        def w8(w):
            return np.ascontiguousarray(
                (_tileP(np.asarray(w, np.float32)) * SW).astype(f8))
        pblob = np.zeros((128, 84), np.float32)
        # LN gain/bias folds: g1 scales Wq/Wk/Wv rows, g2 scales W1 rows;
        # ln1_b enters via Q/K evac biases (V's share is constant across
        # tokens post-softmax, so it folds into bo); b2 -> gelu bias.
        g1v = np.asarray(inputs["ln1_g"], np.float32)
        b1v = np.asarray(inputs["ln1_b"], np.float32)
        g2v = np.asarray(inputs["ln2_g"], np.float32)
        b2v = np.asarray(inputs["ln2_b"], np.float32)
        Wq = np.asarray(inputs["Wq"], np.float32)
        Wk = np.asarray(inputs["Wk"], np.float32)
        Wv = np.asarray(inputs["Wv"], np.float32)
        Wo = np.asarray(inputs["Wo"], np.float32)
        pblob[:, 0:8] = _vec_t(Wq.T @ b1v) / np.sqrt(HD)
        pblob[:, 8:16] = _vec_t(Wk.T @ b1v)
        pblob[:, 32:40] = _vec_t(np.asarray(inputs["bo"], np.float32)
                                 + Wo.T @ (Wv.T @ b1v))
        pblob[:, 40:48] = _vec_t(inputs["b2"])
        pblob[:, 48:80] = (np.asarray(inputs["b1"], np.float32)
                           + np.asarray(inputs["W1"], np.float32).T @ b2v
                           ).reshape(32, 128).T
        def whl(w, gcols):
            # hi/lo fp8 split at one shared scale SW; lo is the residual.
            # Group-blocked [128, ngroups, 32 or 16, gcols] so each group is
            # one contiguous DMA.
            t = _tileP(np.asarray(w, np.float32)) * SW
            hi = t.astype(f8)
            lo = (t - hi.astype(np.float32)).astype(f8)
            hl = np.concatenate([hi, lo], axis=1)       # [128, 2*kt, N]
            ng = hl.shape[2] // gcols
            return np.ascontiguousarray(
                hl.reshape(128, hl.shape[1], ng, gcols).transpose(0, 2, 1, 3))
        shared = {
            "wq": w8(Wq * g1v[:, None]), "wk": w8(Wk * g1v[:, None]),
            "wv": w8(Wv * g1v[:, None]), "wo": w8(Wo),
            "w1hl": whl(np.asarray(inputs["W1"], np.float32)
                        * g2v[:, None], 512),
            "w2hl": whl(inputs["W2"], 128),
            "pblob_base": pblob,
        }
        shared_cache[key] = shared
    x = np.asarray(inputs["x"], np.float32)
    xT = np.ascontiguousarray(
        _make_x_ext(x, c).T.reshape(8, 128, NSLOT)
        .transpose(1, 0, 2)).astype(bf)
    m = _make_mask(c)                                    # [NQB, 128, WIN+1]
    msk = np.ascontiguousarray(
        m[:, :, :WIN].reshape(NQB, 128, 2, 128)
        .transpose(3, 0, 2, 1)).astype(bf)               # [p, qb, blk, q]
    mskg = np.ascontiguousarray(
        m[:, :, WIN].T - 3.0).astype(np.float32)   # token-0 col bias (-3 exp
                                                   # shift folded in)
    fs = np.full((128, 1), 1 if c % 4 == 3 else 0, np.uint8)
    pblob = shared["pblob_base"].copy()
    pblob[0:16, 80] = 1.0 if c < 4 else 0.0
    pblob[0:16, 81] = 0.0 if c < 4 else 1.0
    ret = {k: v for k, v in shared.items() if k != "pblob_base"}
    ret.update({"xT": xT, "msk": msk, "mskg": mskg, "fixsel": fs,
                "pblob": pblob})
    return ret


def get_nc():
    if "nc" not in _CACHE:
        _CACHE["nc"] = _build_nc()
    return _CACHE["nc"]


def kernel(**inputs):
    nc = get_nc()
    in_maps = [_prep_core_inputs(inputs, c) for c in range(N_CORES)]
    res = run_bass_kernel_spmd(nc, in_maps, core_ids=list(range(N_CORES)),
                               trace=False)
    out = np.zeros((B, T, D), np.float32)
    for c in range(N_CORES):
        b, j = divmod(c, 4)
        oT = res.results[c]["outT"]          # [128, 8, 512]
        out[b, j * CHUNK:(j + 1) * CHUNK] = \
            oT.transpose(1, 0, 2).reshape(D, CHUNK).T
    return out

